# revision 13
# baseline (speedup 1.0000x reference)
"""GCNConv (batched dense-adjacency GraphConv) Trainium2 kernel.

Math: out[b] = sum_n relu((A[b] @ X[b]) @ W1 + b1) @ W2 + N * b2
Using (A X) W1 == A (X W1), precompute Y = X @ W1 on host (tiny), so the
device only does the memory-bound part: Z^T = Y^T A^T per batch, then
relu+bias+column-sum on the Activation engine. Host finishes with the
[B,4] @ [4,1] readout.

Per batch the PE streams A^T (the moving operand) once per "stream";
the stationary operand stacks several Y components side by side
([Y_hi | Y_lo], 4 cols each), so one moving pass produces each
component's partial sums in disjoint PSUM rows. The Vector engine adds
the PSUM row-groups, and one ScalarE activation does
relu(. + b1) + free-dim accumulation into the per-batch column sums.

Sharding: pure data parallel over the batch dim, 32 batches per core x 8.
"""

import sys

if "/opt/trn_rl_repo" not in sys.path:
    sys.path.insert(0, "/opt/trn_rl_repo")

import numpy as np

import concourse.bass as bass  # noqa: F401
import concourse.mybir as mybir
import concourse.tile as tile
from concourse import bacc
from concourse.bass_utils import run_bass_kernel_spmd

N_CORES = 8
B, N, F = 256, 512, 2
H = 4  # hidden dim after W1
BPC = B // N_CORES  # batches per core
NCH = N // 128  # contraction chunks per batch

# Each strategy is a list of streams; stream = (a_dtype, a_part, y_comps)
#   a_dtype: dtype of the A^T moving tensor for this stream
#   a_part:  "hi" -> round(A), "lo" -> A - round_as(A, a_dtype of stream 0)
#   y_comps: stationary components stacked in lhsT, each (y_dtype, y_part)
# Error ~ (A rounding of finest stream) x (Y rounding of finest comp).
_F32 = "float32"
_F32R = "float32r"
_BF16 = "bfloat16"
_FP16 = "float16"

STRATEGIES = {
    # exact fp32 (PE at 1/4 rate)
    "fp32": [(_F32, "hi", [(_F32, "hi")])],
    # relaxed-precision full-rate fp32 matmul
    "f32r": [(_F32R, "hi", [(_F32R, "hi")])],
    # plain bf16 / fp16 (half DMA bytes)
    "bf16": [(_BF16, "hi", [(_BF16, "hi")])],
    "fp16": [(_FP16, "hi", [(_FP16, "hi")])],
    # fp16 A, Y split hi+lo (kills the Y-rounding term; same bytes as fp16)
    "fp16s": [(_FP16, "hi", [(_FP16, "hi"), (_FP16, "lo")])],
    # bf16 hi/lo split of A and Y: fp32-class accuracy, fp32 DMA bytes,
    # but only 2 moving passes of the PE per batch
    "bf16x2s": [
        (_BF16, "hi", [(_BF16, "hi"), (_BF16, "lo")]),
        (_BF16, "lo", [(_BF16, "hi")]),
    ],
}

STRATEGY = "fp16s"

_BUILT = {}


def _dt(name):
    return getattr(mybir.dt, name)


def _np_dt(name):
    import ml_dtypes
    return {"float32": np.float32, "float32r": np.float32,
            "bfloat16": ml_dtypes.bfloat16, "float16": np.float16}[name]


def _build(strategy, repeat=1):
    """Build + compile the Bass module (once per process per strategy).

    repeat > 1 wraps the per-batch loop in a device-side For loop that
    re-runs the whole workload `repeat` times — used only for timing
    (amortizes host dispatch overhead over many on-device iterations).
    """
    streams = STRATEGIES[strategy]
    f32 = mybir.dt.float32

    nc = bacc.Bacc("TRN2", target_bir_lowering=False, debug=False,
                   num_devices=N_CORES)

    # at_s[b]: A[b]^T packed [128, NCH*N]: at[b][p][c*N+n] = A_part[b][n][c*128+p]
    ats = [nc.dram_tensor(f"at{s}", [BPC, 128, NCH * N], _dt(a_dt),
                          kind="ExternalInput")
           for s, (a_dt, _, _) in enumerate(streams)]
    # y_s packed [128, BPC*NCH*W_s]; per (b,c) block comp k sits at
    # columns [32*k, 32*k+H) (zeros elsewhere)
    def _w(y_comps):
        return 32 * (len(y_comps) - 1) + H

    ys = [nc.dram_tensor(f"y{s}", [128, BPC * NCH * _w(y_comps)],
                         _dt(y_comps[0][0]), kind="ExternalInput")
          for s, (_, _, y_comps) in enumerate(streams)]
    b1d = nc.dram_tensor("b1", [H, 1], f32, kind="ExternalInput")
    outd = nc.dram_tensor("out", [H, BPC], f32, kind="ExternalOutput")

    # Engine APs must start at a partition base that's a multiple of 32, so
    # stream 0's stacked components live in 32-row PSUM bands (rows 32k..+H,
    # lhsT stacks them 32 columns apart with zero padding). Streams >= 1 must
    # be single-component; they accumulate straight into rows 0..H via the
    # PE's PSUM accumulation (start=False), costing no combine work.
    n_groups = len(streams[0][2])
    for (_, _, y_comps) in streams[1:]:
        assert len(y_comps) == 1, "secondary streams must be single-component"
    n_rows = 32 * (n_groups - 1) + H  # psum tile partition count

    # SBUF budget is ample; buffer enough A tiles to keep DMA queues busy.
    a_bufs = 6

    with tile.TileContext(nc) as tc:
        with tc.tile_pool(name="const", bufs=1) as constp, \
             tc.tile_pool(name="apool", bufs=a_bufs) as apool, \
             tc.tile_pool(name="scratch", bufs=3) as spool, \
             tc.tile_pool(name="psum", bufs=3, space="PSUM") as ppool:
            b1_t = constp.tile([H, 1], f32)
            nc.sync.dma_start(out=b1_t[:], in_=b1d[:])
            y_ts = []
            for s, (_, _, y_comps) in enumerate(streams):
                y_t = constp.tile([128, BPC * NCH * _w(y_comps)],
                                  _dt(y_comps[0][0]), tag=f"y{s}")
                nc.sync.dma_start(out=y_t[:], in_=ys[s][:])
                y_ts.append(y_t)
            out_t = constp.tile([H, BPC], f32)

            def batch_body(b):
                a_ts = []
                for s, (a_dt, _, _) in enumerate(streams):
                    a_t = apool.tile([128, NCH * N], _dt(a_dt), tag=f"a{s}")
                    nc.sync.dma_start(out=a_t[:], in_=ats[s][b])
                    a_ts.append(a_t)
                ps = ppool.tile([n_rows, N], f32)
                last_s = len(streams) - 1
                for s, (_, _, y_comps) in enumerate(streams):
                    w = _w(y_comps)
                    for c in range(NCH):
                        nc.tensor.matmul(
                            ps[0:w, :],
                            y_ts[s][:, (b * NCH + c) * w:(b * NCH + c + 1) * w],
                            a_ts[s][:, c * N:(c + 1) * N],
                            start=(s == 0 and c == 0),
                            stop=(c == NCH - 1 and (s == 0 or s == last_s)),
                            skip_group_check=True,
                        )
                # add stream 0's 32-row-spaced component bands on VectorE
                # (an op may read PSUM through at most one input, so stage
                # the extra band through SBUF first)
                if n_groups == 1:
                    z_ap = ps[0:H, :]
                else:
                    acc = spool.tile([H, N], f32, tag="acc")
                    for gi in range(1, n_groups):
                        tmp = spool.tile([H, N], f32, tag="tmp")
                        nc.vector.tensor_copy(
                            out=tmp[:], in_=ps[32 * gi:32 * gi + H, :])
                        nc.vector.tensor_add(
                            acc[:], ps[0:H, :] if gi == 1 else acc[:], tmp[:])
                    z_ap = acc[:]
                sc = spool.tile([H, N], f32, tag="sc")
                nc.scalar.activation(
                    sc[:], z_ap, mybir.ActivationFunctionType.Relu,
                    bias=b1_t[:], scale=1.0,
                    accum_out=out_t[:, b:b + 1],
                )

            if repeat == 1:
                for b in range(BPC):
                    batch_body(b)
            else:
                with tc.For_i(0, repeat, 1):
                    for b in range(BPC):
                        batch_body(b)
            nc.sync.dma_start(out=outd[:], in_=out_t[:])

    nc.compile()
    return nc


def _get_nc(strategy=None, repeat=1):
    strategy = strategy or STRATEGY
    key = (strategy, repeat)
    if key not in _BUILT:
        _BUILT[key] = _build(strategy, repeat)
    return _BUILT[key]


def _pack_at(adj):
    """[Bc, N, N] f32 -> A^T packed [Bc, 128, NCH*N] (see _build)."""
    t = adj.reshape(adj.shape[0], N, NCH, 128)  # [b, n, c, p]
    return np.ascontiguousarray(t.transpose(0, 3, 2, 1)).reshape(
        adj.shape[0], 128, NCH * N)


def _pack_y(comps):
    """comps: list of [Bc, N, H] f32 arrays -> [128, Bc*NCH*W] with comp k at
    columns [32*k, 32*k+H) of each (b, c) block (zeros elsewhere)."""
    bc = comps[0].shape[0]
    w = 32 * (len(comps) - 1) + H
    out = np.zeros((128, bc, NCH, w), np.float32)
    for k, y in enumerate(comps):
        # y [b, c, p, j] -> [p, b, c, j]
        out[:, :, :, 32 * k:32 * k + H] = y.reshape(
            bc, NCH, 128, H).transpose(2, 0, 1, 3)
    return out.reshape(128, bc * NCH * w)


def _split(full, dt_name):
    """Return (hi, lo) parts of `full` (f32) for the given storage dtype."""
    np_dt = _np_dt(dt_name)
    hi = full.astype(np_dt)
    lo = (full - hi.astype(np.float32)).astype(np_dt)
    return hi, lo


def _prep_in_maps(node_features, adj_matrices, W1, b1, strategy):
    streams = STRATEGIES[strategy]
    y_full = np.einsum("bnf,fh->bnh", node_features, W1).astype(np.float32)
    b1_col = np.asarray(b1, np.float32).reshape(H, 1)
    in_maps = []
    for core in range(N_CORES):
        sl = slice(core * BPC, (core + 1) * BPC)
        at = _pack_at(np.ascontiguousarray(adj_matrices[sl]))
        y_sh = y_full[sl]
        m = {"b1": b1_col}
        a_parts = {}
        for s, (a_dt, a_part, y_comps) in enumerate(streams):
            if (a_dt, a_part) not in a_parts:
                if a_part == "hi":
                    a_parts[(a_dt, "hi")] = at.astype(_np_dt(a_dt))
                else:
                    hi = at.astype(_np_dt(streams[0][0]))
                    a_parts[(a_dt, "lo")] = (
                        at - hi.astype(np.float32)).astype(_np_dt(a_dt))
            m[f"at{s}"] = a_parts[(a_dt, a_part)]
            comps = []
            for (y_dt, y_part) in y_comps:
                hi, lo = _split(y_sh, y_dt)
                comps.append((hi if y_part == "hi" else lo).astype(np.float32))
            m[f"y{s}"] = _pack_y(comps).astype(_np_dt(y_comps[0][0]))
        in_maps.append(m)
    return in_maps


def _finish(results, W2, b2):
    # results[c]["out"]: [H, BPC]; colsum[b, j] = sum_n relu(Z + b1)[n, j]
    cols = np.stack([r["out"] for r in results])  # [8, H, BPC]
    colsum = cols.transpose(0, 2, 1).reshape(B, H).astype(np.float32)
    out = colsum @ np.asarray(W2, np.float32) + N * np.asarray(b2, np.float32)
    return out.astype(np.float32)


def kernel(node_features, adj_matrices, W1, b1, W2, b2):
    node_features = np.asarray(node_features, np.float32)
    adj_matrices = np.asarray(adj_matrices, np.float32)
    nc = _get_nc()
    in_maps = _prep_in_maps(node_features, adj_matrices, W1, b1, STRATEGY)
    res = run_bass_kernel_spmd(nc, in_maps, core_ids=list(range(N_CORES)))
    return _finish(res.results, W2, b2)
